# revision 1
# baseline (speedup 1.0000x reference)
"""Trainium2 Bass kernel: 8192x8192 valid 3x3 cross-correlation + scalar bias.

Strategy: shard rows across 8 NeuronCores (1024 output rows each, with
2-row input halo supplied host-side; no collectives). Per core the conv is
computed on TensorE as 3 PSUM-accumulating float32r matmuls per output
tile: the stationary operand is a banded matrix A_dj[i, io] = w[i-io, dj]
built on host from the 3x3 weight, the moving operand is the X row-strip
shifted by dj columns. PSUM is evicted in 1024-wide (2-bank) groups split
2:1 between DVE tensor_scalar_add and ACT activation(Identity) — both fuse
the bias — then stored with HWDGE DMA.
"""

from contextlib import ExitStack

import numpy as np

import concourse.bass as bass
import concourse.tile as tile
from concourse import bacc, mybir
from concourse.bass_utils import run_bass_kernel_spmd

N_CORES = 8
H = W = 8192
KH = KW = 3
OH, OW = H - KH + 1, W - KW + 1          # 8190, 8190
ROWS_PER_CORE = H // N_CORES             # 1024 output rows per core (last 2 garbage)
IN_ROWS = ROWS_PER_CORE + KH - 1         # 1026 input rows per core
TILE_M = 126                             # output rows per PE tile (K = TILE_M + 2 = 128)
CHUNK = 512                              # PSUM bank = 512 fp32

_cached = {}


CFG = dict(
    xbufs=2,        # input-strip pool buffers
    ybufs=2,        # output-strip pool buffers
    psbufs=4,       # PSUM pool buffers (4 groups x evw banks = all 8 banks)
    load_eng="sync",    # HWDGE ring for loads
    store_eng="scalar",  # HWDGE ring for stores
    pair=0,         # 1: two 126-row strips per DMA (8.4 MB transfers)
    split=1,        # loads split into N column chunks
    split_store=1,  # stores split into N column chunks
    evict="both38", # "dve" | "both": PSUM eviction engine(s)
    evw=2,          # chunks per eviction group (2 = one DVE op per 2 banks)
    first_split=4,  # first strip's load split into N pieces (cold-start ramp)
    mmdt="f32r",    # "f32r" | "bf16": matmul operand dtype
    skip_compute=0,  # diagnostic: no matmuls/DVE (wrong output)
    skip_store=0,    # diagnostic: no output stores (wrong output)
)


def _strided_ap(base_ap, offset, dims):
    """AP over `base_ap`'s tensor with explicit element offset + (step, count) dims."""
    c = base_ap.copy()
    c.offset = offset
    c.ap = type(c.ap)(list(dims))
    return c


def _build_program(reps=1, **overrides):
    cfg = {**CFG, **overrides}
    key = ("nc", reps, tuple(sorted(cfg.items())))
    if key in _cached:
        return _cached[key]

    f32 = mybir.dt.float32
    f32r = mybir.dt.float32r
    bf16 = cfg["mmdt"] == "bf16"
    mmdt = mybir.dt.bfloat16 if bf16 else f32r

    nc = bacc.Bacc("TRN2", target_bir_lowering=False, debug=False,
                   num_devices=N_CORES)
    x_d = nc.dram_tensor("x", [IN_ROWS, W], f32r, kind="ExternalInput")
    a_d = nc.dram_tensor("a", [128, KW, TILE_M], mmdt, kind="ExternalInput")
    b_d = nc.dram_tensor("b", [128, 1], f32, kind="ExternalInput")
    y_d = nc.dram_tensor("y", [ROWS_PER_CORE, OW], f32, kind="ExternalOutput")

    # strip schedule: (out_row0, M) — 8 full tiles of 126 + a 16-row tail
    strips = []
    r = 0
    while r < ROWS_PER_CORE:
        m = min(TILE_M, ROWS_PER_CORE - r)
        strips.append((r, m))
        r += m

    n_chunks = (OW + CHUNK - 1) // CHUNK  # 16 (last = 510)

    with tile.TileContext(nc) as tc, ExitStack() as ctx:
        const_pool = ctx.enter_context(tc.tile_pool(name="const", bufs=1))
        xpool = ctx.enter_context(tc.tile_pool(name="xin", bufs=cfg["xbufs"]))
        ypool = ctx.enter_context(tc.tile_pool(name="yout", bufs=cfg["ybufs"]))
        pspool = ctx.enter_context(
            tc.tile_pool(name="psum", bufs=cfg["psbufs"],
                         space=bass.MemorySpace.PSUM))
        load_eng = getattr(nc, cfg["load_eng"])
        store_eng = getattr(nc, cfg["store_eng"])

        a_s = const_pool.tile([128, KW, TILE_M], mmdt)
        nc.sync.dma_start(a_s[:], a_d.ap())
        b_s = const_pool.tile([128, 1], f32)
        nc.sync.dma_start(b_s[:], b_d.ap())

        def do_chunks(ps_dst, m, k, xs_src, ys_dst):
            """16 output chunks for one strip: 3 matmuls each, eviction per
            group of evw chunks (one DVE op spanning evw PSUM banks)."""
            if cfg["skip_compute"]:
                return
            evw = cfg["evw"]
            for g in range(0, n_chunks, evw):
                gchunks = range(g, min(g + evw, n_chunks))
                gcol0 = g * CHUNK
                gwidth = min((g + evw) * CHUNK, OW) - gcol0
                ps = pspool.tile([128, CHUNK * evw], f32, tag="ps")
                for c in gchunks:
                    col0 = c * CHUNK
                    n = min(CHUNK, OW - col0)
                    po = col0 - gcol0
                    for dj in range(KW):
                        nc.tensor.matmul(
                            ps[:m, po:po + n],
                            a_s[:k, dj, :m],
                            xs_src[:k, col0 + dj:col0 + dj + n],
                            start=(dj == 0),
                            stop=(dj == KW - 1),
                        )
                gi = g // evw
                act_turn = (cfg["evict"] == "both" and gi % 3 == 2) or (
                    cfg["evict"] == "both38" and gi % 8 in (2, 5, 7))
                if act_turn:
                    nc.scalar.activation(
                        ys_dst[:m, gcol0:gcol0 + gwidth], ps[:m, :gwidth],
                        mybir.ActivationFunctionType.Identity,
                        bias=b_s[:m, :], scale=1.0)
                else:
                    nc.vector.tensor_scalar_add(
                        ys_dst[:m, gcol0:gcol0 + gwidth], ps[:m, :gwidth],
                        b_s[:m, :])

        if not cfg["pair"]:
            nsp = cfg["split"]
            for si, (r0, m) in enumerate(
                    [s for _ in range(reps) for s in strips]):
                k = m + KH - 1
                xs = xpool.tile([128, W], mmdt, tag="xs")
                xld = nc.gpsimd if bf16 else load_eng  # SWDGE casts f32->bf16
                # finer pieces for the very first load so PE starts sooner
                nld = cfg["first_split"] if si == 0 else nsp
                for sp in range(nld):
                    c0, c1 = W * sp // nld, W * (sp + 1) // nld
                    xld.dma_start(xs[:k, c0:c1], x_d.ap()[r0:r0 + k, c0:c1])
                if cfg["skip_compute"]:
                    # diagnostic: store straight from xs (wrong output)
                    store_eng.dma_start(y_d.ap()[r0:r0 + m, :],
                                        xs[:m, :OW].bitcast(f32))
                    continue
                ys = ypool.tile([128, OW], f32, tag="ys")
                do_chunks(None, m, k, xs, ys)
                if not cfg["skip_store"]:
                    nss = cfg["split_store"]
                    for sp in range(nss):
                        c0, c1 = OW * sp // nss, OW * (sp + 1) // nss
                        store_eng.dma_start(y_d.ap()[r0:r0 + m, c0:c1],
                                            ys[:m, c0:c1])
        else:
            # pairs of 126-row strips: one 8.4 MB load / 8.3 MB store each
            assert len(strips) == 9
            for _ in range(reps):
                for p in range(4):
                    r0 = strips[2 * p][0]
                    xs = xpool.tile([128, 2, W], f32r, tag="xs")
                    load_eng.dma_start(
                        xs[:],
                        _strided_ap(x_d.ap(), r0 * W,
                                    [(W, 128), (TILE_M * W, 2), (1, W)]))
                    ys = ypool.tile([128, 2, OW], f32, tag="ys")
                    for j in range(2):
                        do_chunks(None, TILE_M, 128, xs[:, j, :], ys[:, j, :])
                    store_eng.dma_start(
                        _strided_ap(y_d.ap(), r0 * OW,
                                    [(OW, TILE_M), (TILE_M * OW, 2), (1, OW)]),
                        ys[:TILE_M, :, :])
                # tail strip (16 rows)
                r0, m = strips[8]
                k = m + KH - 1
                xs = xpool.tile([128, 2, W], f32r, tag="xs")
                load_eng.dma_start(xs[:k, 0, :], x_d.ap()[r0:r0 + k, :])
                ys = ypool.tile([128, 2, OW], f32, tag="ys")
                do_chunks(None, m, k, xs[:, 0, :], ys[:, 0, :])
                store_eng.dma_start(y_d.ap()[r0:r0 + m, :], ys[:m, 0, :])

    nc.compile()
    _cached[key] = nc
    return nc


def _host_inputs(X, weight, bias, mmdt="f32r"):
    """Build the 8 per-core input maps from full inputs."""
    X = np.ascontiguousarray(X, dtype=np.float32)
    weight = np.asarray(weight, dtype=np.float32)
    bias = np.asarray(bias, dtype=np.float32)

    # banded stationary matrices: a[p, dj, io] = weight[p - io, dj]
    a = np.zeros((128, KW, TILE_M), dtype=np.float32)
    for di in range(KH):
        for dj in range(KW):
            for io in range(TILE_M):
                a[io + di, dj, io] = weight[di, dj]

    if mmdt == "bf16":
        import ml_dtypes
        a = a.astype(ml_dtypes.bfloat16)

    b = np.full((128, 1), bias[0], dtype=np.float32)

    # core 7 needs input rows up to 8193; pad 2 zero rows (its last 2
    # output rows are garbage and trimmed on unshard)
    Xpad = np.concatenate([X, np.zeros((2, W), dtype=np.float32)], axis=0)

    in_maps = []
    for c in range(N_CORES):
        r0 = c * ROWS_PER_CORE
        in_maps.append({
            "x": np.ascontiguousarray(Xpad[r0:r0 + IN_ROWS]),
            "a": a,
            "b": b,
        })
    return in_maps


def kernel(X, weight, bias):
    nc = _build_program()
    in_maps = _host_inputs(X, weight, bias)
    res = run_bass_kernel_spmd(nc, in_maps, core_ids=list(range(N_CORES)))
    out = np.concatenate([res.results[c]["y"] for c in range(N_CORES)], axis=0)
    return out[:OH]



# revision 3
# speedup vs baseline: 4.3841x; 4.3841x over previous
"""Trainium2 Bass kernel: 8192x8192 valid 3x3 cross-correlation + scalar bias.

Strategy: shard rows across 8 NeuronCores (1024 output rows each, with
2-row input halo supplied host-side; no collectives). Per core the conv is
computed on TensorE as 3 PSUM-accumulating float32r matmuls per output
tile: the stationary operand is a banded matrix A_dj[i, io] = w[i-io, dj]
built on host from the 3x3 weight, the moving operand is the X row-strip
shifted by dj columns. PSUM is evicted in 1024-wide (2-bank) groups split
2:1 between DVE tensor_scalar_add and ACT activation(Identity) — both fuse
the bias — then stored with HWDGE DMA.
"""

from contextlib import ExitStack

import numpy as np

import concourse.bass as bass
import concourse.tile as tile
from concourse import bacc, mybir
from concourse.bass_utils import run_bass_kernel_spmd

N_CORES = 8
H = W = 8192
KH = KW = 3
OH, OW = H - KH + 1, W - KW + 1          # 8190, 8190
ROWS_PER_CORE = H // N_CORES             # 1024 output rows per core (last 2 garbage)
IN_ROWS = ROWS_PER_CORE + KH - 1         # 1026 input rows per core
TILE_M = 126                             # output rows per PE tile (K = TILE_M + 2 = 128)
CHUNK = 512                              # PSUM bank = 512 fp32

_cached = {}


CFG = dict(
    xbufs=2,        # input-strip pool buffers
    ybufs=2,        # output-strip pool buffers
    psbufs=4,       # PSUM pool buffers (4 groups x evw banks = all 8 banks)
    load_eng="sync",    # HWDGE ring for loads
    store_eng="scalar",  # HWDGE ring for stores
    pair=0,         # 1: two 126-row strips per DMA (8.4 MB transfers)
    split=1,        # loads split into N column chunks
    split_store=1,  # stores split into N column chunks
    evict="both38", # "dve" | "both": PSUM eviction engine(s)
    evw=2,          # chunks per eviction group (2 = one DVE op per 2 banks)
    first_split=4,  # first strip's load split into N pieces (cold-start ramp)
    mmdt="f32r",    # "f32r" | "bf16": matmul operand dtype
    skip_compute=0,  # diagnostic: no matmuls/DVE (wrong output)
    skip_store=0,    # diagnostic: no output stores (wrong output)
)


def _strided_ap(base_ap, offset, dims):
    """AP over `base_ap`'s tensor with explicit element offset + (step, count) dims."""
    c = base_ap.copy()
    c.offset = offset
    c.ap = type(c.ap)(list(dims))
    return c


def _build_program(reps=1, hwreps=1, **overrides):
    cfg = {**CFG, **overrides}
    key = ("nc", reps, hwreps, tuple(sorted(cfg.items())))
    if key in _cached:
        return _cached[key]

    f32 = mybir.dt.float32
    f32r = mybir.dt.float32r
    bf16 = cfg["mmdt"] == "bf16"
    mmdt = mybir.dt.bfloat16 if bf16 else f32r

    nc = bacc.Bacc("TRN2", target_bir_lowering=False, debug=False,
                   num_devices=N_CORES)
    x_d = nc.dram_tensor("x", [IN_ROWS, W], f32r, kind="ExternalInput")
    a_d = nc.dram_tensor("a", [128, KW, TILE_M], mmdt, kind="ExternalInput")
    b_d = nc.dram_tensor("b", [128, 1], f32, kind="ExternalInput")
    y_d = nc.dram_tensor("y", [ROWS_PER_CORE, OW], f32, kind="ExternalOutput")

    # strip schedule: (out_row0, M) — 8 full tiles of 126 + a 16-row tail
    strips = []
    r = 0
    while r < ROWS_PER_CORE:
        m = min(TILE_M, ROWS_PER_CORE - r)
        strips.append((r, m))
        r += m

    n_chunks = (OW + CHUNK - 1) // CHUNK  # 16 (last = 510)

    with tile.TileContext(nc) as tc, ExitStack() as ctx:
        const_pool = ctx.enter_context(tc.tile_pool(name="const", bufs=1))
        xpool = ctx.enter_context(tc.tile_pool(name="xin", bufs=cfg["xbufs"]))
        ypool = ctx.enter_context(tc.tile_pool(name="yout", bufs=cfg["ybufs"]))
        pspool = ctx.enter_context(
            tc.tile_pool(name="psum", bufs=cfg["psbufs"],
                         space=bass.MemorySpace.PSUM))
        load_eng = getattr(nc, cfg["load_eng"])
        store_eng = getattr(nc, cfg["store_eng"])

        a_s = const_pool.tile([128, KW, TILE_M], mmdt)
        nc.sync.dma_start(a_s[:], a_d.ap())
        b_s = const_pool.tile([128, 1], f32)
        nc.sync.dma_start(b_s[:], b_d.ap())

        def do_chunks(ps_dst, m, k, xs_src, ys_dst):
            """16 output chunks for one strip: 3 matmuls each, eviction per
            group of evw chunks (one DVE op spanning evw PSUM banks)."""
            if cfg["skip_compute"]:
                return
            evw = cfg["evw"]
            for g in range(0, n_chunks, evw):
                gchunks = range(g, min(g + evw, n_chunks))
                gcol0 = g * CHUNK
                gwidth = min((g + evw) * CHUNK, OW) - gcol0
                ps = pspool.tile([128, CHUNK * evw], f32, tag="ps")
                for c in gchunks:
                    col0 = c * CHUNK
                    n = min(CHUNK, OW - col0)
                    po = col0 - gcol0
                    for dj in range(KW):
                        nc.tensor.matmul(
                            ps[:m, po:po + n],
                            a_s[:k, dj, :m],
                            xs_src[:k, col0 + dj:col0 + dj + n],
                            start=(dj == 0),
                            stop=(dj == KW - 1),
                        )
                gi = g // evw
                act_turn = (cfg["evict"] == "both" and gi % 3 == 2) or (
                    cfg["evict"] == "both38" and gi % 8 in (2, 5, 7))
                if act_turn:
                    nc.scalar.activation(
                        ys_dst[:m, gcol0:gcol0 + gwidth], ps[:m, :gwidth],
                        mybir.ActivationFunctionType.Identity,
                        bias=b_s[:m, :], scale=1.0)
                else:
                    nc.vector.tensor_scalar_add(
                        ys_dst[:m, gcol0:gcol0 + gwidth], ps[:m, :gwidth],
                        b_s[:m, :])

        def emit_schedule():
            nsp = cfg["split"]
            for si, (r0, m) in enumerate(
                    [s for _ in range(reps) for s in strips]):
                k = m + KH - 1
                xs = xpool.tile([128, W], mmdt, tag="xs")
                xld = nc.gpsimd if bf16 else load_eng  # SWDGE casts f32->bf16
                # finer pieces for the very first load so PE starts sooner
                nld = cfg["first_split"] if si == 0 else nsp
                for sp in range(nld):
                    c0, c1 = W * sp // nld, W * (sp + 1) // nld
                    xld.dma_start(xs[:k, c0:c1], x_d.ap()[r0:r0 + k, c0:c1])
                if cfg["skip_compute"]:
                    # diagnostic: store straight from xs (wrong output)
                    store_eng.dma_start(y_d.ap()[r0:r0 + m, :],
                                        xs[:m, :OW].bitcast(f32))
                    continue
                ys = ypool.tile([128, OW], f32, tag="ys")
                do_chunks(None, m, k, xs, ys)
                if not cfg["skip_store"]:
                    nss = cfg["split_store"]
                    for sp in range(nss):
                        c0, c1 = OW * sp // nss, OW * (sp + 1) // nss
                        store_eng.dma_start(y_d.ap()[r0:r0 + m, c0:c1],
                                            ys[:m, c0:c1])

        if not cfg["pair"]:
            if hwreps > 1:
                with tc.For_i(0, hwreps):
                    emit_schedule()
            else:
                emit_schedule()
        else:
            # pairs of 126-row strips: one 8.4 MB load / 8.3 MB store each
            assert len(strips) == 9
            for _ in range(reps):
                for p in range(4):
                    r0 = strips[2 * p][0]
                    xs = xpool.tile([128, 2, W], f32r, tag="xs")
                    load_eng.dma_start(
                        xs[:],
                        _strided_ap(x_d.ap(), r0 * W,
                                    [(W, 128), (TILE_M * W, 2), (1, W)]))
                    ys = ypool.tile([128, 2, OW], f32, tag="ys")
                    for j in range(2):
                        do_chunks(None, TILE_M, 128, xs[:, j, :], ys[:, j, :])
                    store_eng.dma_start(
                        _strided_ap(y_d.ap(), r0 * OW,
                                    [(OW, TILE_M), (TILE_M * OW, 2), (1, OW)]),
                        ys[:TILE_M, :, :])
                # tail strip (16 rows)
                r0, m = strips[8]
                k = m + KH - 1
                xs = xpool.tile([128, 2, W], f32r, tag="xs")
                load_eng.dma_start(xs[:k, 0, :], x_d.ap()[r0:r0 + k, :])
                ys = ypool.tile([128, 2, OW], f32, tag="ys")
                do_chunks(None, m, k, xs[:, 0, :], ys[:, 0, :])
                store_eng.dma_start(y_d.ap()[r0:r0 + m, :], ys[:m, 0, :])

    nc.compile()
    _cached[key] = nc
    return nc


def _host_inputs(X, weight, bias, mmdt="f32r"):
    """Build the 8 per-core input maps from full inputs."""
    X = np.ascontiguousarray(X, dtype=np.float32)
    weight = np.asarray(weight, dtype=np.float32)
    bias = np.asarray(bias, dtype=np.float32)

    # banded stationary matrices: a[p, dj, io] = weight[p - io, dj]
    a = np.zeros((128, KW, TILE_M), dtype=np.float32)
    for di in range(KH):
        for dj in range(KW):
            for io in range(TILE_M):
                a[io + di, dj, io] = weight[di, dj]

    if mmdt == "bf16":
        import ml_dtypes
        a = a.astype(ml_dtypes.bfloat16)

    b = np.full((128, 1), bias[0], dtype=np.float32)

    # core 7 needs input rows up to 8193; pad 2 zero rows (its last 2
    # output rows are garbage and trimmed on unshard)
    Xpad = np.concatenate([X, np.zeros((2, W), dtype=np.float32)], axis=0)

    in_maps = []
    for c in range(N_CORES):
        r0 = c * ROWS_PER_CORE
        in_maps.append({
            "x": np.ascontiguousarray(Xpad[r0:r0 + IN_ROWS]),
            "a": a,
            "b": b,
        })
    return in_maps


def kernel(X, weight, bias):
    nc = _build_program()
    in_maps = _host_inputs(X, weight, bias)
    res = run_bass_kernel_spmd(nc, in_maps, core_ids=list(range(N_CORES)))
    out = np.concatenate([res.results[c]["y"] for c in range(N_CORES)], axis=0)
    return out[:OH]



# revision 6
# speedup vs baseline: 5.5022x; 1.2550x over previous
"""Trainium2 Bass kernel: 8192x8192 valid 3x3 cross-correlation + scalar bias.

Strategy: shard rows across 8 NeuronCores (1024 output rows each, with
2-row input halo supplied host-side; no collectives). Per core the conv is
computed on TensorE as 3 PSUM-accumulating float32r matmuls per output
tile: the stationary operand is a banded matrix A_dj[i, io] = w[i-io, dj]
built on host from the 3x3 weight, the moving operand is the X row-strip
shifted by dj columns. PSUM is evicted in 1024-wide (2-bank) groups split
2:1 between DVE tensor_scalar_add and ACT activation(Identity) — both fuse
the bias — then stored with HWDGE DMA.
"""

from contextlib import ExitStack

import numpy as np

import concourse.bass as bass
import concourse.tile as tile
from concourse import bacc, mybir
from concourse.bass_utils import run_bass_kernel_spmd

N_CORES = 8
H = W = 8192
KH = KW = 3
OH, OW = H - KH + 1, W - KW + 1          # 8190, 8190
ROWS_PER_CORE = H // N_CORES             # 1024 output rows per core (last 2 garbage)
IN_ROWS = ROWS_PER_CORE + KH - 1         # 1026 input rows per core
TILE_M = 126                             # output rows per PE tile (K = TILE_M + 2 = 128)
CHUNK = 512                              # PSUM bank = 512 fp32

_cached = {}


CFG = dict(
    xbufs=3,        # input-strip pool buffers
    ybufs=3,        # output-strip pool buffers
    psbufs=4,       # PSUM pool buffers (4 groups x evw banks = all 8 banks)
    load_eng="sync",    # HWDGE ring(s) for loads (comma list round-robins)
    store_eng="scalar",  # HWDGE ring(s) for stores
    pair=0,         # 1: two 126-row strips per DMA (8.4 MB transfers)
    split=1,        # loads split into N column chunks
    split_store=2,  # stores split into N column chunks
    evict="both",   # "dve" | "both" | "both38": PSUM eviction engine(s)
    evw=2,          # chunks per eviction group (2 = one DVE op per 2 banks)
    first_split=8,  # first strip's load split into N pieces (cold-start ramp)
    mmdt="f32r",    # "f32r" | "bf16": matmul operand dtype
    skip_compute=0,  # diagnostic: no matmuls/DVE (wrong output)
    skip_store=0,    # diagnostic: no output stores (wrong output)
)


def _strided_ap(base_ap, offset, dims):
    """AP over `base_ap`'s tensor with explicit element offset + (step, count) dims."""
    c = base_ap.copy()
    c.offset = offset
    c.ap = type(c.ap)(list(dims))
    return c


def _build_program(reps=1, hwreps=1, **overrides):
    cfg = {**CFG, **overrides}
    key = ("nc", reps, hwreps, tuple(sorted(cfg.items())))
    if key in _cached:
        return _cached[key]

    f32 = mybir.dt.float32
    f32r = mybir.dt.float32r
    bf16 = cfg["mmdt"] == "bf16"
    mmdt = mybir.dt.bfloat16 if bf16 else f32r

    nc = bacc.Bacc("TRN2", target_bir_lowering=False, debug=False,
                   num_devices=N_CORES)
    x_d = nc.dram_tensor("x", [IN_ROWS, W], f32r, kind="ExternalInput")
    a_d = nc.dram_tensor("a", [128, KW, TILE_M], mmdt, kind="ExternalInput")
    b_d = nc.dram_tensor("b", [128, 1], f32, kind="ExternalInput")
    y_d = nc.dram_tensor("y", [ROWS_PER_CORE, OW], f32, kind="ExternalOutput")

    # strip schedule: (out_row0, M) — 8 full tiles of 126 + a 16-row tail
    strips = []
    r = 0
    while r < ROWS_PER_CORE:
        m = min(TILE_M, ROWS_PER_CORE - r)
        strips.append((r, m))
        r += m

    n_chunks = (OW + CHUNK - 1) // CHUNK  # 16 (last = 510)

    with tile.TileContext(nc) as tc, ExitStack() as ctx:
        const_pool = ctx.enter_context(tc.tile_pool(name="const", bufs=1))
        xpool = ctx.enter_context(tc.tile_pool(name="xin", bufs=cfg["xbufs"]))
        ypool = ctx.enter_context(tc.tile_pool(name="yout", bufs=cfg["ybufs"]))
        pspool = ctx.enter_context(
            tc.tile_pool(name="psum", bufs=cfg["psbufs"],
                         space=bass.MemorySpace.PSUM))
        load_rings = [getattr(nc, e) for e in cfg["load_eng"].split(",")]
        store_rings = [getattr(nc, e) for e in cfg["store_eng"].split(",")]
        ring_idx = [0, 0]

        class _RR:
            """Round-robin DMA ring selector (cycles per dma_start call)."""
            def __init__(self, rings, slot):
                self.rings, self.slot = rings, slot

            def dma_start(self, *a, **k):
                r = self.rings[ring_idx[self.slot] % len(self.rings)]
                ring_idx[self.slot] += 1
                return r.dma_start(*a, **k)

        load_eng = _RR(load_rings, 0)
        store_eng = _RR(store_rings, 1)

        a_s = const_pool.tile([128, KW, TILE_M], mmdt)
        nc.sync.dma_start(a_s[:], a_d.ap())
        b_s = const_pool.tile([128, 1], f32)
        nc.sync.dma_start(b_s[:], b_d.ap())

        def do_chunks(ps_dst, m, k, xs_src, ys_dst):
            """16 output chunks for one strip: 3 matmuls each, eviction per
            group of evw chunks (one DVE op spanning evw PSUM banks)."""
            if cfg["skip_compute"]:
                return
            evw = cfg["evw"]
            for g in range(0, n_chunks, evw):
                gchunks = range(g, min(g + evw, n_chunks))
                gcol0 = g * CHUNK
                gwidth = min((g + evw) * CHUNK, OW) - gcol0
                ps = pspool.tile([128, CHUNK * evw], f32, tag="ps")
                for c in gchunks:
                    col0 = c * CHUNK
                    n = min(CHUNK, OW - col0)
                    po = col0 - gcol0
                    for dj in range(KW):
                        nc.tensor.matmul(
                            ps[:m, po:po + n],
                            a_s[:k, dj, :m],
                            xs_src[:k, col0 + dj:col0 + dj + n],
                            start=(dj == 0),
                            stop=(dj == KW - 1),
                        )
                gi = g // evw
                act_turn = (cfg["evict"] == "both" and gi % 3 == 2) or (
                    cfg["evict"] == "both38" and gi % 8 in (2, 5, 7))
                if act_turn:
                    nc.scalar.activation(
                        ys_dst[:m, gcol0:gcol0 + gwidth], ps[:m, :gwidth],
                        mybir.ActivationFunctionType.Identity,
                        bias=b_s[:m, :], scale=1.0)
                else:
                    nc.vector.tensor_scalar_add(
                        ys_dst[:m, gcol0:gcol0 + gwidth], ps[:m, :gwidth],
                        b_s[:m, :])

        def emit_schedule():
            nsp = cfg["split"]
            for si, (r0, m) in enumerate(
                    [s for _ in range(reps) for s in strips]):
                k = m + KH - 1
                xs = xpool.tile([128, W], mmdt, tag="xs")
                xld = nc.gpsimd if bf16 else load_eng  # SWDGE casts f32->bf16
                # finer pieces for the very first load so PE starts sooner
                nld = cfg["first_split"] if si == 0 else nsp
                for sp in range(nld):
                    c0, c1 = W * sp // nld, W * (sp + 1) // nld
                    xld.dma_start(xs[:k, c0:c1], x_d.ap()[r0:r0 + k, c0:c1])
                if cfg["skip_compute"]:
                    # diagnostic: store straight from xs (wrong output)
                    if not cfg["skip_store"]:
                        store_eng.dma_start(y_d.ap()[r0:r0 + m, :],
                                            xs[:m, :OW].bitcast(f32))
                    continue
                ys = ypool.tile([128, OW], f32, tag="ys")
                do_chunks(None, m, k, xs, ys)
                if not cfg["skip_store"]:
                    nss = cfg["split_store"]
                    for sp in range(nss):
                        c0, c1 = OW * sp // nss, OW * (sp + 1) // nss
                        store_eng.dma_start(y_d.ap()[r0:r0 + m, c0:c1],
                                            ys[:m, c0:c1])

        if not cfg["pair"]:
            if hwreps > 1:
                with tc.For_i(0, hwreps):
                    emit_schedule()
            else:
                emit_schedule()
        else:
            # pairs of 126-row strips: one 8.4 MB load / 8.3 MB store each
            assert len(strips) == 9
            for _ in range(reps):
                for p in range(4):
                    r0 = strips[2 * p][0]
                    xs = xpool.tile([128, 2, W], f32r, tag="xs")
                    load_eng.dma_start(
                        xs[:],
                        _strided_ap(x_d.ap(), r0 * W,
                                    [(W, 128), (TILE_M * W, 2), (1, W)]))
                    ys = ypool.tile([128, 2, OW], f32, tag="ys")
                    for j in range(2):
                        do_chunks(None, TILE_M, 128, xs[:, j, :], ys[:, j, :])
                    store_eng.dma_start(
                        _strided_ap(y_d.ap(), r0 * OW,
                                    [(OW, TILE_M), (TILE_M * OW, 2), (1, OW)]),
                        ys[:TILE_M, :, :])
                # tail strip (16 rows)
                r0, m = strips[8]
                k = m + KH - 1
                xs = xpool.tile([128, 2, W], f32r, tag="xs")
                load_eng.dma_start(xs[:k, 0, :], x_d.ap()[r0:r0 + k, :])
                ys = ypool.tile([128, 2, OW], f32, tag="ys")
                do_chunks(None, m, k, xs[:, 0, :], ys[:, 0, :])
                store_eng.dma_start(y_d.ap()[r0:r0 + m, :], ys[:m, 0, :])

    nc.compile()
    _cached[key] = nc
    return nc


def _host_inputs(X, weight, bias, mmdt="f32r"):
    """Build the 8 per-core input maps from full inputs."""
    X = np.ascontiguousarray(X, dtype=np.float32)
    weight = np.asarray(weight, dtype=np.float32)
    bias = np.asarray(bias, dtype=np.float32)

    # banded stationary matrices: a[p, dj, io] = weight[p - io, dj]
    a = np.zeros((128, KW, TILE_M), dtype=np.float32)
    for di in range(KH):
        for dj in range(KW):
            for io in range(TILE_M):
                a[io + di, dj, io] = weight[di, dj]

    if mmdt == "bf16":
        import ml_dtypes
        a = a.astype(ml_dtypes.bfloat16)

    b = np.full((128, 1), bias[0], dtype=np.float32)

    # core 7 needs input rows up to 8193; pad 2 zero rows (its last 2
    # output rows are garbage and trimmed on unshard)
    Xpad = np.concatenate([X, np.zeros((2, W), dtype=np.float32)], axis=0)

    in_maps = []
    for c in range(N_CORES):
        r0 = c * ROWS_PER_CORE
        in_maps.append({
            "x": np.ascontiguousarray(Xpad[r0:r0 + IN_ROWS]),
            "a": a,
            "b": b,
        })
    return in_maps


def kernel(X, weight, bias):
    nc = _build_program()
    in_maps = _host_inputs(X, weight, bias)
    res = run_bass_kernel_spmd(nc, in_maps, core_ids=list(range(N_CORES)))
    out = np.concatenate([res.results[c]["y"] for c in range(N_CORES)], axis=0)
    return out[:OH]



# revision 8
# speedup vs baseline: 5.5695x; 1.0122x over previous
"""Trainium2 Bass kernel: 8192x8192 valid 3x3 cross-correlation + scalar bias.

Strategy: shard rows across 8 NeuronCores (1024 output rows each, with
2-row input halo supplied host-side; no collectives). Per core the conv is
computed on TensorE as 3 PSUM-accumulating float32r matmuls per output
tile: the stationary operand is a banded matrix A_dj[i, io] = w[i-io, dj]
built on host from the 3x3 weight, the moving operand is the X row-strip
shifted by dj columns. PSUM is evicted in 1024-wide (2-bank) groups split
2:1 between DVE tensor_scalar_add and ACT activation(Identity) — both fuse
the bias — then stored with HWDGE DMA.

The kernel is purely DMA-bound (measured: loads+stores-only time equals the
full kernel; ~67.7 MB/core at the per-NC HBM path rate). Tuning therefore
targets DMA duty cycle: triple-buffered input/output strips (loads run 3
strips ahead), stores split in column halves so each half fires as soon as
its eviction groups land, loads on the sync HWDGE ring and stores on the
scalar ring (sharing one ring serializes stores' sem-waits ahead of loads),
and the first strip's load split 8x so the PE pipeline primes during the
cold ramp. Timeline-sim: DMA busy is gapless from 3.3 us to the last store.
"""

from contextlib import ExitStack

import numpy as np

import concourse.bass as bass
import concourse.tile as tile
from concourse import bacc, mybir
from concourse.bass_utils import run_bass_kernel_spmd

N_CORES = 8
H = W = 8192
KH = KW = 3
OH, OW = H - KH + 1, W - KW + 1          # 8190, 8190
ROWS_PER_CORE = H // N_CORES             # 1024 output rows per core (last 2 garbage)
IN_ROWS = ROWS_PER_CORE + KH - 1         # 1026 input rows per core
TILE_M = 126                             # output rows per PE tile (K = TILE_M + 2 = 128)
CHUNK = 512                              # PSUM bank = 512 fp32

_cached = {}


CFG = dict(
    xbufs=3,        # input-strip pool buffers
    ybufs=3,        # output-strip pool buffers
    psbufs=4,       # PSUM pool buffers (4 groups x evw banks = all 8 banks)
    load_eng="sync",    # HWDGE ring(s) for loads (comma list round-robins)
    store_eng="scalar",  # HWDGE ring(s) for stores
    pair=0,         # 1: two 126-row strips per DMA (8.4 MB transfers)
    split=1,        # loads split into N column chunks
    split_store=2,  # stores split into N column chunks
    evict="both",   # "dve" | "both" | "both38": PSUM eviction engine(s)
    evw=2,          # chunks per eviction group (2 = one DVE op per 2 banks)
    first_split=8,  # first strip's load split into N pieces (cold-start ramp)
    mmdt="f32r",    # "f32r" | "bf16": matmul operand dtype
    skip_compute=0,  # diagnostic: no matmuls/DVE (wrong output)
    skip_store=0,    # diagnostic: no output stores (wrong output)
)


def _strided_ap(base_ap, offset, dims):
    """AP over `base_ap`'s tensor with explicit element offset + (step, count) dims."""
    c = base_ap.copy()
    c.offset = offset
    c.ap = type(c.ap)(list(dims))
    return c


def _build_program(reps=1, hwreps=1, **overrides):
    cfg = {**CFG, **overrides}
    key = ("nc", reps, hwreps, tuple(sorted(cfg.items())))
    if key in _cached:
        return _cached[key]

    f32 = mybir.dt.float32
    f32r = mybir.dt.float32r
    bf16 = cfg["mmdt"] == "bf16"
    mmdt = mybir.dt.bfloat16 if bf16 else f32r

    nc = bacc.Bacc("TRN2", target_bir_lowering=False, debug=False,
                   num_devices=N_CORES)
    x_d = nc.dram_tensor("x", [IN_ROWS, W], f32r, kind="ExternalInput")
    a_d = nc.dram_tensor("a", [128, KW, TILE_M], mmdt, kind="ExternalInput")
    b_d = nc.dram_tensor("b", [128, 1], f32, kind="ExternalInput")
    y_d = nc.dram_tensor("y", [ROWS_PER_CORE, OW], f32, kind="ExternalOutput")

    # strip schedule: (out_row0, M) — 8 full tiles of 126 + a 16-row tail
    strips = []
    r = 0
    while r < ROWS_PER_CORE:
        m = min(TILE_M, ROWS_PER_CORE - r)
        strips.append((r, m))
        r += m

    n_chunks = (OW + CHUNK - 1) // CHUNK  # 16 (last = 510)

    with tile.TileContext(nc) as tc, ExitStack() as ctx:
        const_pool = ctx.enter_context(tc.tile_pool(name="const", bufs=1))
        xpool = ctx.enter_context(tc.tile_pool(name="xin", bufs=cfg["xbufs"]))
        ypool = ctx.enter_context(tc.tile_pool(name="yout", bufs=cfg["ybufs"]))
        pspool = ctx.enter_context(
            tc.tile_pool(name="psum", bufs=cfg["psbufs"],
                         space=bass.MemorySpace.PSUM))
        load_rings = [getattr(nc, e) for e in cfg["load_eng"].split(",")]
        store_rings = [getattr(nc, e) for e in cfg["store_eng"].split(",")]
        ring_idx = [0, 0]

        class _RR:
            """Round-robin DMA ring selector (cycles per dma_start call)."""
            def __init__(self, rings, slot):
                self.rings, self.slot = rings, slot

            def dma_start(self, *a, **k):
                r = self.rings[ring_idx[self.slot] % len(self.rings)]
                ring_idx[self.slot] += 1
                return r.dma_start(*a, **k)

        load_eng = _RR(load_rings, 0)
        store_eng = _RR(store_rings, 1)

        # const loads ride the store ring (idle at head) so they don't delay
        # the first x-strip load on the sync ring
        const_eng = getattr(nc, cfg.get("const_eng", "scalar"))
        a_s = const_pool.tile([128, KW, TILE_M], mmdt)
        const_eng.dma_start(a_s[:], a_d.ap())
        b_s = const_pool.tile([128, 1], f32)
        const_eng.dma_start(b_s[:], b_d.ap())

        def do_chunks(ps_dst, m, k, xs_src, ys_dst):
            """16 output chunks for one strip: 3 matmuls each, eviction per
            group of evw chunks (one DVE op spanning evw PSUM banks)."""
            if cfg["skip_compute"]:
                return
            evw = cfg["evw"]
            for g in range(0, n_chunks, evw):
                gchunks = range(g, min(g + evw, n_chunks))
                gcol0 = g * CHUNK
                gwidth = min((g + evw) * CHUNK, OW) - gcol0
                ps = pspool.tile([128, CHUNK * evw], f32, tag="ps")
                for c in gchunks:
                    col0 = c * CHUNK
                    n = min(CHUNK, OW - col0)
                    po = col0 - gcol0
                    for dj in range(KW):
                        nc.tensor.matmul(
                            ps[:m, po:po + n],
                            a_s[:k, dj, :m],
                            xs_src[:k, col0 + dj:col0 + dj + n],
                            start=(dj == 0),
                            stop=(dj == KW - 1),
                        )
                gi = g // evw
                act_turn = (cfg["evict"] == "both" and gi % 3 == 2) or (
                    cfg["evict"] == "both38" and gi % 8 in (2, 5, 7))
                if act_turn:
                    nc.scalar.activation(
                        ys_dst[:m, gcol0:gcol0 + gwidth], ps[:m, :gwidth],
                        mybir.ActivationFunctionType.Identity,
                        bias=b_s[:m, :], scale=1.0)
                else:
                    nc.vector.tensor_scalar_add(
                        ys_dst[:m, gcol0:gcol0 + gwidth], ps[:m, :gwidth],
                        b_s[:m, :])

        def emit_schedule():
            nsp = cfg["split"]
            for si, (r0, m) in enumerate(
                    [s for _ in range(reps) for s in strips]):
                k = m + KH - 1
                xs = xpool.tile([128, W], mmdt, tag="xs")
                xld = nc.gpsimd if bf16 else load_eng  # SWDGE casts f32->bf16
                # finer pieces for the very first load so PE starts sooner
                nld = cfg["first_split"] if si == 0 else nsp
                for sp in range(nld):
                    c0, c1 = W * sp // nld, W * (sp + 1) // nld
                    xld.dma_start(xs[:k, c0:c1], x_d.ap()[r0:r0 + k, c0:c1])
                if cfg["skip_compute"]:
                    # diagnostic: store straight from xs (wrong output)
                    if not cfg["skip_store"]:
                        store_eng.dma_start(y_d.ap()[r0:r0 + m, :],
                                            xs[:m, :OW].bitcast(f32))
                    continue
                ys = ypool.tile([128, OW], f32, tag="ys")
                do_chunks(None, m, k, xs, ys)
                if not cfg["skip_store"]:
                    nss = cfg["split_store"]
                    for sp in range(nss):
                        c0, c1 = OW * sp // nss, OW * (sp + 1) // nss
                        store_eng.dma_start(y_d.ap()[r0:r0 + m, c0:c1],
                                            ys[:m, c0:c1])

        if not cfg["pair"]:
            if hwreps > 1:
                with tc.For_i(0, hwreps):
                    emit_schedule()
            else:
                emit_schedule()
        else:
            # pairs of 126-row strips: one 8.4 MB load / 8.3 MB store each
            assert len(strips) == 9
            for _ in range(reps):
                for p in range(4):
                    r0 = strips[2 * p][0]
                    xs = xpool.tile([128, 2, W], f32r, tag="xs")
                    load_eng.dma_start(
                        xs[:],
                        _strided_ap(x_d.ap(), r0 * W,
                                    [(W, 128), (TILE_M * W, 2), (1, W)]))
                    ys = ypool.tile([128, 2, OW], f32, tag="ys")
                    for j in range(2):
                        do_chunks(None, TILE_M, 128, xs[:, j, :], ys[:, j, :])
                    store_eng.dma_start(
                        _strided_ap(y_d.ap(), r0 * OW,
                                    [(OW, TILE_M), (TILE_M * OW, 2), (1, OW)]),
                        ys[:TILE_M, :, :])
                # tail strip (16 rows)
                r0, m = strips[8]
                k = m + KH - 1
                xs = xpool.tile([128, 2, W], f32r, tag="xs")
                load_eng.dma_start(xs[:k, 0, :], x_d.ap()[r0:r0 + k, :])
                ys = ypool.tile([128, 2, OW], f32, tag="ys")
                do_chunks(None, m, k, xs[:, 0, :], ys[:, 0, :])
                store_eng.dma_start(y_d.ap()[r0:r0 + m, :], ys[:m, 0, :])

    nc.compile()
    _cached[key] = nc
    return nc


def _host_inputs(X, weight, bias, mmdt="f32r"):
    """Build the 8 per-core input maps from full inputs."""
    X = np.ascontiguousarray(X, dtype=np.float32)
    weight = np.asarray(weight, dtype=np.float32)
    bias = np.asarray(bias, dtype=np.float32)

    # banded stationary matrices: a[p, dj, io] = weight[p - io, dj]
    a = np.zeros((128, KW, TILE_M), dtype=np.float32)
    for di in range(KH):
        for dj in range(KW):
            for io in range(TILE_M):
                a[io + di, dj, io] = weight[di, dj]

    if mmdt == "bf16":
        import ml_dtypes
        a = a.astype(ml_dtypes.bfloat16)

    b = np.full((128, 1), bias[0], dtype=np.float32)

    # core 7 needs input rows up to 8193; pad 2 zero rows (its last 2
    # output rows are garbage and trimmed on unshard)
    Xpad = np.concatenate([X, np.zeros((2, W), dtype=np.float32)], axis=0)

    in_maps = []
    for c in range(N_CORES):
        r0 = c * ROWS_PER_CORE
        in_maps.append({
            "x": np.ascontiguousarray(Xpad[r0:r0 + IN_ROWS]),
            "a": a,
            "b": b,
        })
    return in_maps


def kernel(X, weight, bias):
    nc = _build_program()
    in_maps = _host_inputs(X, weight, bias)
    res = run_bass_kernel_spmd(nc, in_maps, core_ids=list(range(N_CORES)))
    out = np.concatenate([res.results[c]["y"] for c in range(N_CORES)], axis=0)
    return out[:OH]

